# revision 5
# baseline (speedup 1.0000x reference)
"""GAT encoder (PyG GATConv-style, single head) for Trainium2, 8 NeuronCores.

Two-launch design (see kernel_v2 docstring for the derivation):

  Phase 1: node-partitioned projection through W_ext = [W | W@att_src |
  W@att_dst]; emits per node h (32, bf16) and exp factors u=e^{a_s},
  p=e^{0.2 a_s}, v=e^{a_d}, q=e^{0.2 a_d} (ACT engine).
  exp(leaky_relu(a_s+a_d)) == max(u*v, p*q), so phase 2 needs no
  transcendentals and no per-dst segment max (logits are bounded).

  Host (pure indexing): gathers h / u / p by edge source and v / q by edge
  destination into a dst-major slot stream.

  Phase 2: per GROUP of tiles (variable T chosen by a DP that trades slot
  padding against per-instruction overhead): stream h [128, T, 32, S] (slot
  dim innermost keeps DVE in 2x 16-bit mode), num = max(u*v, p*q) (DVE,
  chunked), den = ACT Copy+accum_out per tile, msg = h * num_bcast and slot
  tree-fold split between DVE [0,Sd) and GpSimd/Pool [Sd,S) writing
  independent partials (merged in the batched epilogue with 1/den, +bias,
  ACT sigmoid).

Destinations are partitioned contiguously across cores and degree-sorted
into 128-dst tiles so each group shares a tight slot count S.
Precision: bf16 streams, f32 den; rel err ~5e-3 (fp8 h measured 2.1e-2 >
the 2e-2 gate, hence bf16).
"""
import os
import sys

for _p in ('/opt/trn_rl_repo',):
    if _p not in sys.path and os.path.isdir(_p):
        sys.path.insert(0, _p)

import numpy as np
import ml_dtypes

import concourse.mybir as mybir
import concourse.tile as tile
from concourse import bacc
from concourse.bass_utils import run_bass_kernel_spmd

F32 = mybir.dt.float32
BF16 = mybir.dt.bfloat16
NPBF16 = ml_dtypes.bfloat16

N_CORES = 8
PSUM_CHUNK = 15          # 15*34 = 510 <= 512 f32 per PSUM bank
CW = 34                  # projected width: 32 h + a_s + a_d
POOL_FRAC = float(os.environ.get("GAT_POOL", "0.25"))  # slot share on GpSimd
TCAP = int(os.environ.get("GAT_TCAP", "10"))           # max tiles per group
DP_RATE = 0.418 * 64     # ~ns per slot of mult+fold at combined DVE+Pool rate
DP_TAX = 640.0           # ~ns of per-group instruction overhead

LAST_RESULTS = None
_NC_CACHE = {}
_LAST_NCS = ()


def sim_exec_time_ns():
    """Sum of TimelineSim estimates for the programs run by kernel()."""
    from concourse.timeline_sim import TimelineSim
    return int(sum(TimelineSim(nc, trace=False).simulate()
                   for nc in _LAST_NCS))


# ---------------------------------------------------------------- planning
def _plan(dst, N, n_cores):
    """Degree-sorted 128-dst tiles per core + DP tile grouping.

    Returns (Nc, n_tiles, groups, cores) where groups is a list of
    (tile0, T, S) in emission order and cores[c] = (eidx_sorted, counts,
    offsets, d_pad) with d_pad following tile emission order.
    """
    Nc = N // n_cores
    assert Nc * n_cores == N
    n_tiles = -(-Nc // 128)
    S_tile = np.zeros(n_tiles, np.int64)
    cores = []
    for c in range(n_cores):
        sel = (dst >= c * Nc) & (dst < (c + 1) * Nc)
        idx = np.nonzero(sel)[0]
        d_c = dst[idx] - c * Nc
        order = np.argsort(d_c, kind='stable')
        eidx_sorted = idx[order]
        counts = np.bincount(d_c, minlength=Nc).astype(np.int64)
        offsets = np.zeros(Nc + 1, np.int64)
        np.cumsum(counts, out=offsets[1:])
        perm = np.argsort(-counts, kind='stable')
        cnt_sorted = np.zeros(n_tiles * 128, np.int64)
        cnt_sorted[:Nc] = counts[perm]
        S_c = np.maximum(cnt_sorted.reshape(n_tiles, 128).max(axis=1), 1)
        S_tile = np.maximum(S_tile, S_c)
        d_pad = np.full(n_tiles * 128, Nc, np.int64)
        d_pad[:Nc] = perm
        cores.append((eidx_sorted, counts, offsets, d_pad))
    # DP grouping of consecutive (degree-sorted) tiles
    INF = 1e18
    best = np.full(n_tiles + 1, INF)
    best[n_tiles] = 0.0
    choice = np.zeros(n_tiles, np.int64)
    for i in range(n_tiles - 1, -1, -1):
        for T in range(1, min(TCAP, n_tiles - i) + 1):
            mx = int(max(S_tile[i:i + T]))
            cst = DP_RATE * T * mx + DP_TAX + best[i + T]
            if cst < best[i]:
                best[i] = cst
                choice[i] = T
    groups = []
    i = 0
    while i < n_tiles:
        T = int(choice[i])
        groups.append((i, T, int(max(S_tile[i:i + T]))))
        i += T
    # emission order: two smallest (by elems) first, rest descending,
    # smallest of the rest last
    key = [T * S for (_, T, S) in groups]
    asc = sorted(range(len(groups)), key=lambda g: key[g])
    if len(asc) > 3:
        emit = asc[:2] + sorted(asc[3:], key=lambda g: -key[g]) + [asc[2]]
    else:
        emit = asc
    groups = [groups[g] for g in emit]
    # d_pad reordered to follow emission tile order
    tile_order = np.concatenate(
        [np.arange(t0, t0 + T) for (t0, T, _) in groups])
    for c in range(n_cores):
        e, cnt, off, d_pad = cores[c]
        d_pad = d_pad.reshape(n_tiles, 128)[tile_order].reshape(-1)
        cores[c] = (e, cnt, off, d_pad)
    groups = [(T, S) for (_, T, S) in groups]
    return Nc, n_tiles, groups, cores


def _entries(core_plan, groups, n_edges):
    """Per-group [T, S, 128] edge-id tables; id n_edges = padding slot."""
    eidx_sorted, counts, offsets, d_pad = core_plan
    e_pad = np.concatenate([eidx_sorted, [n_edges]])
    counts_p = np.concatenate([counts, [0]])
    offsets_p = np.concatenate([offsets, [len(eidx_sorted)]])
    ents = []
    t0 = 0
    for (T, S) in groups:
        d = d_pad[t0 * 128:(t0 + T) * 128].reshape(T, 128)
        k = np.arange(S)
        cnt = counts_p[d]
        pos = offsets_p[d][:, None, :] + k[None, :, None]
        valid = k[None, :, None] < cnt[:, None, :]
        ent = np.full((T, S, 128), len(e_pad) - 1, np.int64)
        ent[valid] = np.minimum(pos[valid], len(e_pad) - 1)
        ents.append(e_pad[ent])
        t0 += T
    return ents


# ---------------------------------------------------------------- phase 1
def _build_proj(nch):
    nc = bacc.Bacc("TRN2", target_bir_lowering=False, debug=False,
                   num_devices=N_CORES)
    xt = nc.dram_tensor("xt", [128, nch * 128], BF16, kind="ExternalInput").ap()
    wext = nc.dram_tensor("wext", [128, CW], BF16, kind="ExternalInput").ap()
    h_out = nc.dram_tensor("h_out", [128, nch * 32], BF16,
                           kind="ExternalOutput").ap()
    e1_out = nc.dram_tensor("e1_out", [128, nch * 2], BF16,
                            kind="ExternalOutput").ap()
    e2_out = nc.dram_tensor("e2_out", [128, nch * 2], BF16,
                            kind="ExternalOutput").ap()
    with tile.TileContext(nc) as tc:
        with (
            tc.tile_pool(name="const", bufs=1) as cpool,
            tc.tile_pool(name="xc", bufs=4) as xpool,
            tc.tile_pool(name="ps", bufs=8, space="PSUM") as pspool,
        ):
            wsb = cpool.tile([128, CW], BF16)
            nc.sync.dma_start(wsb[:], wext[:])
            hsb = cpool.tile([128, nch * 32], BF16)
            asd = cpool.tile([128, nch * 2], F32)
            b0 = 0
            while b0 < nch:
                cn = min(PSUM_CHUNK, nch - b0)
                xc = xpool.tile([128, PSUM_CHUNK * 128], BF16, tag="xc")
                nc.sync.dma_start(xc[:, :cn * 128],
                                  xt[:, b0 * 128:(b0 + cn) * 128])
                ps = pspool.tile([128, PSUM_CHUNK * CW], F32, tag="ps")
                for j in range(cn):
                    nc.tensor.matmul(
                        ps[:, j * CW:(j + 1) * CW],
                        xc[:, j * 128:(j + 1) * 128],
                        wsb[:], start=True, stop=True)
                psv = ps[:, :cn * CW].rearrange("p (s f) -> p s f", f=CW)
                nc.vector.tensor_copy(
                    out=hsb[:, b0 * 32:(b0 + cn) * 32]
                    .rearrange("p (s c) -> p s c", c=32),
                    in_=psv[:, :, 0:32])
                nc.vector.tensor_copy(
                    out=asd[:, b0 * 2:(b0 + cn) * 2]
                    .rearrange("p (s c) -> p s c", c=2),
                    in_=psv[:, :, 32:34])
                # h out on the other HWDGE queue (ACT) so descriptor
                # generation overlaps the xt input stream (SP)
                nc.scalar.dma_start(h_out[:, b0 * 32:(b0 + cn) * 32],
                                    hsb[:, b0 * 32:(b0 + cn) * 32])
                b0 += cn
            e1 = cpool.tile([128, nch * 2], BF16)
            e2 = cpool.tile([128, nch * 2], BF16)
            nc.scalar.activation(e1[:], asd[:],
                                 mybir.ActivationFunctionType.Exp, scale=1.0)
            nc.scalar.activation(e2[:], asd[:],
                                 mybir.ActivationFunctionType.Exp, scale=0.2)
            nc.scalar.dma_start(e1_out[:], e1[:])
            nc.sync.dma_start(e2_out[:], e2[:])
    nc.compile()
    return nc


def _chunk(sizes, n_chunks):
    """Split indices into <=n_chunks contiguous spans of ~equal weight."""
    total = sum(sizes)
    bounds, acc, r0 = [], 0, 0
    for i, s in enumerate(sizes):
        acc += s
        if acc >= total / n_chunks * (len(bounds) + 1) or i == len(sizes) - 1:
            bounds.append((r0, i + 1))
            r0 = i + 1
    return bounds


def _chunks_front(sizes):
    """A tiny first chunk (first 2 groups) so num for the pipeline head is
    ready fast, then ~5 equal chunks for the rest."""
    if len(sizes) <= 2:
        return [(0, len(sizes))]
    return [(0, 2)] + [(a + 2, b + 2) for a, b in _chunk(sizes[2:], 5)]


# ---------------------------------------------------------------- phase 2
def _build_agg(groups, n_tiles, pool_frac):
    nslots = int(sum(T * S for (T, S) in groups))          # per partition
    htot = nslots * 32
    nc = bacc.Bacc("TRN2", target_bir_lowering=False, debug=False,
                   num_devices=N_CORES)
    he = nc.dram_tensor("he", [128, htot], BF16, kind="ExternalInput").ap()
    uvpq = nc.dram_tensor("uvpq", [128, 4 * nslots], BF16,
                          kind="ExternalInput").ap()
    bias = nc.dram_tensor("bias", [128, 32], BF16, kind="ExternalInput").ap()
    out = nc.dram_tensor("out", [128, n_tiles * 32], F32,
                         kind="ExternalOutput").ap()
    maxTS = max(T * S for (T, S) in groups)
    with tile.TileContext(nc) as tc:
        with (
            tc.tile_pool(name="const", bufs=1) as cpool,
            tc.tile_pool(name="st", bufs=4) as stpool,
            tc.tile_pool(name="work", bufs=3) as wpool,
            tc.tile_pool(name="small", bufs=4) as spool,
        ):
            upq = cpool.tile([128, 4 * nslots], BF16)
            num_a = cpool.tile([128, nslots], BF16)
            t2_a = cpool.tile([128, nslots], BF16)
            grp_slots = [T * S for (T, S) in groups]
            chunks = _chunks_front(grp_slots)
            soff = 0
            for ci, (g0, g1) in enumerate(chunks):
                ln = sum(grp_slots[g0:g1])
                base4 = 4 * soff
                nc.sync.dma_start(upq[:, base4:base4 + 4 * ln],
                                  uvpq[:, base4:base4 + 4 * ln])
                u_a = upq[:, base4:base4 + ln]
                v_a = upq[:, base4 + ln:base4 + 2 * ln]
                p_a = upq[:, base4 + 2 * ln:base4 + 3 * ln]
                q_a = upq[:, base4 + 3 * ln:base4 + 4 * ln]
                nv = num_a[:, soff:soff + ln]
                t2v = t2_a[:, soff:soff + ln]
                # pq-mult rides Pool after the first chunk (DVE critical
                # path at startup is uvpq->num->first mult)
                nc.vector.tensor_tensor(out=nv, in0=u_a, in1=v_a,
                                        op=mybir.AluOpType.mult)
                eng = nc.vector if ci == 0 else nc.gpsimd
                eng.tensor_tensor(out=t2v, in0=p_a, in1=q_a,
                                  op=mybir.AluOpType.mult)
                nc.vector.tensor_tensor(out=nv, in0=nv, in1=t2v,
                                        op=mybir.AluOpType.max)
                soff += ln
            bias_sb = cpool.tile([128, 32], BF16)
            nc.sync.dma_start(bias_sb[:], bias[:])
            outp = cpool.tile([128, n_tiles * 32], BF16)
            outp2 = cpool.tile([128, n_tiles * 32], BF16)  # Pool partials
            nc.gpsimd.memset(outp2[:], 0.0)
            den_all = cpool.tile([128, n_tiles], F32)

            # epilogue bounds: ~quarters of tiles + a small final piece
            cum, qb = 0, []
            marks = {n_tiles // 4, n_tiles // 2, (3 * n_tiles) // 4}
            for gi, (T, S) in enumerate(groups):
                cum += T
                if (any(cum >= m > cum - T for m in marks)
                        or gi >= len(groups) - 2):
                    qb.append(gi)
            qb = sorted(set(qb) | {len(groups) - 1})

            base = 0   # h stream offset
            sbase = 0  # slot offset
            tbase = 0  # tile offset
            q0t, q0s = 0, 0   # epilogue window start (tiles, groups)
            for gi, (T, S) in enumerate(groups):
                L = T * 32 * S
                st = stpool.tile([128, maxTS * 32], BF16, tag="st")
                nc.sync.dma_start(st[:, :L], he[:, base:base + L])
                base += L
                h3 = st[:, :L].rearrange("p (t c k) -> p t c k", c=32, k=S)
                nv = num_a[:, sbase:sbase + T * S]
                sbase += T * S
                # den on the (otherwise idle) ACT engine: Copy + accum_out
                nsc = spool.tile([128, maxTS], BF16, tag="nsc")
                for tt in range(T):
                    nc.scalar.activation(
                        nsc[:, tt * S:(tt + 1) * S],
                        nv[:, tt * S:(tt + 1) * S],
                        mybir.ActivationFunctionType.Copy,
                        accum_out=den_all[:, tbase + tt:tbase + tt + 1])
                msg = wpool.tile([128, maxTS * 32], BF16, tag="msg")
                mv = msg[:, :L].rearrange("p (t c k) -> p t c k", c=32, k=S)
                nb = nv.rearrange("p (t k) -> p t k", k=S) \
                    .rearrange("p t (o k) -> p t o k", o=1) \
                    .to_broadcast([128, T, 32, S])
                ov = outp[:, tbase * 32:(tbase + T) * 32] \
                    .rearrange("p (t c) -> p t c", c=32)
                ov2 = outp2[:, tbase * 32:(tbase + T) * 32] \
                    .rearrange("p (t c) -> p t c", c=32)
                Sd = S if S < 6 else max(4, min(S, round(S * (1 - pool_frac))))
                nc.vector.tensor_tensor(out=mv[:, :, :, 0:Sd],
                                        in0=h3[:, :, :, 0:Sd],
                                        in1=nb[:, :, :, 0:Sd],
                                        op=mybir.AluOpType.mult)
                pool_on = Sd < S
                if pool_on:
                    nc.gpsimd.tensor_tensor(out=mv[:, :, :, Sd:S],
                                            in0=h3[:, :, :, Sd:S],
                                            in1=nb[:, :, :, Sd:S],
                                            op=mybir.AluOpType.mult)
                    cur = S - Sd
                    while cur > 2:
                        half = cur // 2
                        nc.gpsimd.tensor_tensor(
                            out=mv[:, :, :, Sd:Sd + half],
                            in0=mv[:, :, :, Sd:Sd + half],
                            in1=mv[:, :, :, Sd + cur - half:Sd + cur],
                            op=mybir.AluOpType.add)
                        cur -= half
                    if cur == 2:
                        nc.gpsimd.tensor_tensor(
                            out=ov2, in0=mv[:, :, :, Sd],
                            in1=mv[:, :, :, Sd + 1], op=mybir.AluOpType.add)
                    else:
                        nc.gpsimd.tensor_copy(out=ov2, in_=mv[:, :, :, Sd])
                if S == 1:
                    nc.vector.tensor_copy(out=ov, in_=mv[:, :, :, 0])
                else:
                    cur = Sd
                    while cur > 2:
                        half = cur // 2
                        nc.vector.tensor_tensor(
                            out=mv[:, :, :, 0:half], in0=mv[:, :, :, 0:half],
                            in1=mv[:, :, :, cur - half:cur],
                            op=mybir.AluOpType.add)
                        cur -= half
                    if cur == 2:
                        nc.vector.tensor_tensor(
                            out=ov, in0=mv[:, :, :, 0], in1=mv[:, :, :, 1],
                            op=mybir.AluOpType.add)
                    else:
                        nc.vector.tensor_copy(out=ov, in_=mv[:, :, :, 0])
                tbase += T

                if gi in qb:
                    nq = tbase - q0t
                    assert nq <= 48, nq
                    dsl = slice(q0t, tbase)
                    osl = slice(q0t * 32, tbase * 32)
                    rb = spool.tile([128, 48], BF16, tag="rb")
                    rcb = rb[:, :nq]
                    # no zero-guard: den==0 only on padding dsts, discarded
                    with nc.allow_low_precision(reason="1/den bf16, 0.4%"):
                        nc.vector.reciprocal(rcb, den_all[:, dsl])
                    ovq = outp[:, osl].rearrange("p (t c) -> p t c", c=32)
                    nc.vector.tensor_tensor(
                        out=ovq, in0=ovq,
                        in1=outp2[:, osl].rearrange("p (t c) -> p t c", c=32),
                        op=mybir.AluOpType.add)
                    nc.vector.tensor_tensor(
                        out=ovq, in0=ovq,
                        in1=rcb.rearrange("p (t o) -> p t o", o=1)
                        .to_broadcast([128, nq, 32]),
                        op=mybir.AluOpType.mult)
                    nc.vector.tensor_tensor(
                        out=ovq, in0=ovq,
                        in1=bias_sb[:].rearrange("p (o c) -> p o c", o=1)
                        .to_broadcast([128, nq, 32]),
                        op=mybir.AluOpType.add)
                    sg = spool.tile([128, 1536], F32, tag="sg")
                    sgv = sg[:, :nq * 32]
                    nc.scalar.activation(sgv, outp[:, osl],
                                         mybir.ActivationFunctionType.Sigmoid)
                    nc.sync.dma_start(out[:, osl], sgv)
                    q0t = tbase
    nc.compile()
    return nc


# ---------------------------------------------------------------- runners
def _run(nc, in_maps, trace):
    if os.environ.get("GAT_SIM"):
        from concourse.bass_interp import CoreSim
        results = []
        for m in in_maps:
            sim = CoreSim(nc, require_finite=False, require_nnan=False)
            for k, v in m.items():
                sim.tensor(k)[:] = v
            sim.simulate()
            outs = {}
            for alloc in nc.m.functions[0].allocations:
                if getattr(alloc, 'kind', None) == "ExternalOutput":
                    name = alloc.memorylocations[0].name
                    outs[name] = np.array(sim.tensor(name))
            results.append(outs)
        class R: pass
        r = R(); r.results = results; r.exec_time_ns = None
        return r
    return run_bass_kernel_spmd(nc, in_maps, core_ids=list(range(N_CORES)),
                                trace=trace)


def kernel(x, edge_index, W, att_src, att_dst, bias):
    global LAST_RESULTS, _LAST_NCS
    x = np.asarray(x, np.float32)
    edge_index = np.asarray(edge_index)
    W = np.asarray(W, np.float32)
    att_src = np.asarray(att_src, np.float32)
    att_dst = np.asarray(att_dst, np.float32)
    bias_np = np.asarray(bias, np.float32)

    N, C_in = x.shape
    C_out = W.shape[1]
    assert C_in == 128 and C_out == 32, (C_in, C_out)
    trace = bool(os.environ.get("GAT_TRACE"))

    loops = np.arange(N, dtype=np.int64)
    src = np.concatenate([edge_index[0].astype(np.int64), loops])
    dst = np.concatenate([edge_index[1].astype(np.int64), loops])
    E = len(src)

    Nc, n_tiles, groups, cores = _plan(dst, N, N_CORES)
    nch = -(-Nc // 128)

    # ---- phase 1: project every node once (node-partitioned) ----
    ws = W @ att_src
    wd = W @ att_dst
    wext = np.concatenate([W, ws[:, None], wd[:, None]], 1).astype(NPBF16)
    in1 = []
    for c in range(N_CORES):
        xt = np.zeros((128, nch * 128), NPBF16)
        n0 = c * Nc
        xt[:, :Nc] = x[n0:n0 + Nc].T.astype(NPBF16)
        in1.append({"xt": xt, "wext": wext})

    key1 = ("proj", nch)
    if key1 not in _NC_CACHE:
        _NC_CACHE[key1] = _build_proj(nch)
    nc1 = _NC_CACHE[key1]
    res1 = _run(nc1, in1, trace)

    # assemble per-node tables; layout [128, nch, K]: node = chunk*128 + p
    h_all = np.zeros((N, 32), NPBF16)
    u_all = np.zeros(N, NPBF16); v_all = np.zeros(N, NPBF16)
    p_all = np.zeros(N, NPBF16); q_all = np.zeros(N, NPBF16)
    for c in range(N_CORES):
        o = res1.results[c]
        h = np.asarray(o["h_out"]).reshape(128, nch, 32) \
            .transpose(1, 0, 2).reshape(nch * 128, 32)[:Nc]
        e1 = np.asarray(o["e1_out"]).reshape(128, nch, 2) \
            .transpose(1, 0, 2).reshape(nch * 128, 2)[:Nc]
        e2 = np.asarray(o["e2_out"]).reshape(128, nch, 2) \
            .transpose(1, 0, 2).reshape(nch * 128, 2)[:Nc]
        sl = slice(c * Nc, (c + 1) * Nc)
        h_all[sl] = h
        u_all[sl], v_all[sl] = e1[:, 0], e1[:, 1]
        p_all[sl], q_all[sl] = e2[:, 0], e2[:, 1]

    # ---- host gather: per-core expansion streams (pure indexing) ----
    hs = np.concatenate([h_all[src], np.zeros((1, 32), NPBF16)], 0)
    us = np.concatenate([u_all[src], [NPBF16(0)]])
    ps_ = np.concatenate([p_all[src], [NPBF16(0)]])
    v_pad = np.concatenate([v_all, [NPBF16(1)]])
    q_pad = np.concatenate([q_all, [NPBF16(1)]])

    nslots = int(sum(T * S for (T, S) in groups))
    htot = nslots * 32
    bias_b = np.broadcast_to(bias_np.astype(NPBF16), (128, 32)).copy()
    grp_slots = [T * S for (T, S) in groups]
    chunks = _chunks_front(grp_slots)
    in2, dpads = [], []
    for c in range(N_CORES):
        ents = _entries(cores[c], groups, E)
        d_pad = cores[c][3]
        he = np.empty((128, htot), NPBF16)
        uv = np.empty((128, 4 * nslots), NPBF16)
        hoff = 0
        gidx = 0
        soff = 0
        for (g0, g1) in chunks:
            glen = sum(grp_slots[g0:g1])
            base4 = 4 * soff
            woff = 0
            tb = sum(T for (T, _) in groups[:g0])
            for gi in range(g0, g1):
                T, S = groups[gi]
                ent = ents[gi]
                he[:, hoff:hoff + T * 32 * S].reshape(
                    128, T, 32, S)[:] = hs[ent].transpose(2, 0, 3, 1)
                uv[:, base4 + woff:base4 + woff + T * S] = \
                    us[ent].transpose(2, 0, 1).reshape(128, -1)
                uv[:, base4 + 2 * glen + woff:
                   base4 + 2 * glen + woff + T * S] = \
                    ps_[ent].transpose(2, 0, 1).reshape(128, -1)
                dl = d_pad[tb * 128:(tb + T) * 128].reshape(T, 128)
                dg = np.where(dl < Nc, dl + c * Nc, N)
                uv[:, base4 + glen + woff:base4 + glen + woff + T * S] = \
                    np.repeat(v_pad[dg].T[:, :, None], S, 2).reshape(128, -1)
                uv[:, base4 + 3 * glen + woff:
                   base4 + 3 * glen + woff + T * S] = \
                    np.repeat(q_pad[dg].T[:, :, None], S, 2).reshape(128, -1)
                hoff += T * 32 * S
                woff += T * S
                tb += T
            soff += glen
        in2.append({"he": he, "uvpq": uv, "bias": bias_b})
        dpads.append(d_pad)

    key2 = ("agg", POOL_FRAC, tuple(groups))
    if key2 not in _NC_CACHE:
        _NC_CACHE[key2] = _build_agg(groups, n_tiles, POOL_FRAC)
    nc2 = _NC_CACHE[key2]
    res2 = _run(nc2, in2, trace)
    LAST_RESULTS = (res1, res2)
    _LAST_NCS = (nc1, nc2)

    out_full = np.zeros((N, C_out), np.float32)
    for c in range(N_CORES):
        o = np.asarray(res2.results[c]["out"]).reshape(128, n_tiles, 32) \
            .transpose(1, 0, 2).reshape(n_tiles * 128, 32)
        d_pad = dpads[c]
        real = d_pad < Nc
        out_full[c * Nc + d_pad[real]] = o[real]
    return out_full
